# Initial kernel scaffold
#
"""Trainium2 Bass kernel for DigitConvolutionalModel:
    out = relu(conv2d_3x3_valid(x.reshape(B,28,28))) .reshape(B,676) @ W + b

Strategy (pure data parallel over 8 cores, B=32768 -> 4096/core):

Per core, samples are processed in 8 "supergroups" (SG) of 512 = 4 groups
(g) of 128. The input DMA loads each group's 128 images in a
"row-on-partition" layout: SBUF X[32g + r, (b, c)] = x[b, 28r + c]
(28 image rows on partitions at 32-aligned bases, batch x column in the
free dim; 112-byte contiguous runs).

Conv (cross-correlation) becomes 3 PSUM-accumulated PE matmuls per
column-chunk: with the host-built Toeplitz matrices
M_dj[r, i] = conv_w[r-i, dj] (28x26, zero elsewhere),
    Y[i, (c, b)] = sum_dj  M_dj^T  @  X[:, (c+dj, b)]
which contracts the row dim r on partitions. Output lands already
"transposed" (output-row i on partitions, (c, b) in the free dim, c-major
so each c gives 128 contiguous batch columns). The small K=28/M=26
matmuls are packed into 32x32 PE tiles: group g reads row-group 32g and
writes col-group 32*((g+q)%4) for column-chunk q, spreading work over all
16 tile positions.

ReLU copies PSUM->SBUF (alternating DVE/ACT), giving H[32cg + i, ...]
with h transposed per sample. The FC layer then contracts i per column c:
    out^T[o, b] += W_c^T @ relu(Y)[:, (c, b)],   W_c[i, o] = W[26i+c, o]
as 26 PSUM-accumulated K=26/M=10/N=256 matmuls (fp32r full rate needs
N>=256, so FC runs per *pair* of supergroups). Bias is added on DVE with
a per-partition scalar, and small PE transposes flip out^T[10,128] tiles
into [128,10] for a contiguous store.
"""

import sys
import numpy as np

for _p in ("/opt/trn_rl_repo", "/root/.axon_site/_ro/trn_rl_repo"):
    if _p not in sys.path:
        sys.path.insert(0, _p)

import concourse.bass as bass  # noqa: E402
import concourse.tile as tile  # noqa: E402
from concourse import bacc, mybir  # noqa: E402
from concourse.bass_utils import run_bass_kernel_spmd  # noqa: E402

IMG = 28
KW = 3
OUT = 26  # IMG - KW + 1
NPIX = IMG * IMG          # 784
NOUTPIX = OUT * OUT       # 676
NCLS = 10
NCORES = 8
B_TOTAL = 32768
B_CORE = B_TOTAL // NCORES   # 4096
SG = 512                     # samples per supergroup (4 groups of 128)
N_SG = B_CORE // SG          # 8
NQ = 7                       # column chunks: 6x4 + 1x2 = 26 columns
F32 = mybir.dt.float32
F32R = mybir.dt.float32r

_CACHE = {}


def _chunk_cols(q):
    """(first output column, n columns) of chunk q."""
    return 4 * q, (2 if q == NQ - 1 else 4)


def _build_program(mm_dtype=F32R):
    """Build + compile the per-core Bass program (identical on all cores)."""
    nc = bacc.Bacc("TRN2", target_bir_lowering=False, debug=False,
                   num_devices=NCORES)

    x_d = nc.dram_tensor("x", (B_CORE, NPIX), F32, kind="ExternalInput")
    msb_d = nc.dram_tensor("msb", (128, 3 * OUT), F32, kind="ExternalInput")
    wsb_d = nc.dram_tensor("wsb", (128, OUT * NCLS), F32, kind="ExternalInput")
    bias_d = nc.dram_tensor("biasv", (128, 1), F32, kind="ExternalInput")
    id_d = nc.dram_tensor("ident", (128, 32), F32, kind="ExternalInput")
    out_d = nc.dram_tensor("out", (B_CORE, NCLS), F32, kind="ExternalOutput")

    x_ap = x_d.ap()
    out_ap = out_d.ap()

    with tile.TileContext(nc) as tc:
        with (
            tc.tile_pool(name="consts", bufs=1) as consts,
            tc.tile_pool(name="xin", bufs=2) as xin,
            tc.tile_pool(name="hbuf", bufs=2) as hbuf,
            tc.tile_pool(name="obuf", bufs=2) as obuf,
            tc.tile_pool(name="convps", bufs=4, space="PSUM") as convps,
            tc.tile_pool(name="fcps", bufs=2, space="PSUM") as fcps,
            tc.tile_pool(name="tps", bufs=2, space="PSUM") as tps,
        ):
            msb = consts.tile([128, 3 * OUT], F32)
            wsb = consts.tile([128, OUT * NCLS], F32)
            biasv = consts.tile([128, 1], F32)
            ident = consts.tile([128, 32], F32)
            nc.sync.dma_start(out=msb[:, :], in_=msb_d.ap())
            nc.sync.dma_start(out=wsb[:, :], in_=wsb_d.ap())
            nc.sync.dma_start(out=biasv[:, :], in_=bias_d.ap())
            nc.sync.dma_start(out=ident[:, :], in_=id_d.ap())

            h_pair = None
            for s in range(N_SG):
                # ---- load supergroup s: 4 groups of 128 samples ----
                xt = xin.tile([128, 128 * IMG], F32, tag="xt")
                for g in range(4):
                    row0 = s * SG + g * 128
                    src = x_ap[row0:row0 + 128, :].rearrange(
                        "b (r c) -> r b c", r=IMG)
                    dst = xt[32 * g:32 * g + IMG, :].rearrange(
                        "p (b c) -> p b c", c=IMG)
                    nc.sync.dma_start(out=dst, in_=src)

                if s % 2 == 0:
                    h_pair = hbuf.tile([128, 2 * NOUTPIX * 2], F32, tag="h")
                    # free layout: s_half*3328 + c*128 + b   (3328 = 26*128)
                half = s % 2

                # ---- conv: per column-chunk q, 4 groups x 3 taps ----
                for q in range(NQ):
                    c0, ncol = _chunk_cols(q)
                    pq = convps.tile([128, 512], F32, tag="pq")
                    xv = [
                        xt[32 * g:32 * g + IMG, :].rearrange(
                            "p (b c) -> p c b", c=IMG)
                        for g in range(4)
                    ]
                    for g in range(4):
                        cg = (g + q) % 4
                        for dj in range(3):
                            nc.tensor.matmul(
                                pq[32 * cg:32 * cg + OUT, 0:ncol * 128],
                                msb[32 * g:32 * g + IMG,
                                    OUT * dj:OUT * dj + OUT].bitcast(mm_dtype),
                                xv[g][:, c0 + dj:c0 + dj + ncol, :].bitcast(
                                    mm_dtype),
                                start=(dj == 0), stop=(dj == 2),
                                tile_position=(32 * g, 32 * cg),
                            )
                    # ---- relu PSUM -> SBUF (h transposed, c-major) ----
                    hslice = h_pair[:, half * 2 * NOUTPIX + c0 * 128:
                                    half * 2 * NOUTPIX + (c0 + ncol) * 128]
                    if (s * NQ + q) % 2 == 0:
                        nc.vector.tensor_scalar_max(
                            hslice, pq[:, 0:ncol * 128], 0.0)
                    else:
                        nc.scalar.activation(
                            hslice, pq[:, 0:ncol * 128],
                            mybir.ActivationFunctionType.Relu)

                # ---- FC + bias + transpose + store, per SG pair ----
                if s % 2 == 1:
                    t = s // 2
                    ot = fcps.tile([128, 256], F32, tag="ot")
                    hv = [
                        h_pair[32 * rg:32 * rg + OUT, :].rearrange(
                            "p (s2 c b) -> p c s2 b", s2=2, b=128)
                        for rg in range(4)
                    ]
                    for g in range(4):
                        for c in range(OUT):
                            rg = (g + c // 4) % 4
                            nc.tensor.matmul(
                                ot[32 * g:32 * g + NCLS, 0:256],
                                wsb[32 * rg:32 * rg + OUT,
                                    NCLS * c:NCLS * c + NCLS].bitcast(mm_dtype),
                                hv[rg][:, c, :, :].bitcast(mm_dtype),
                                start=(c == 0), stop=(c == OUT - 1),
                                tile_position=(32 * rg, 32 * g),
                            )
                    # bias add (per-partition scalar) PSUM -> SBUF
                    osb = obuf.tile([128, 256], F32, tag="osb")
                    nc.vector.tensor_scalar_add(osb[:, :], ot[:, :], biasv[:, 0:1])
                    # transpose out^T [10,128] tiles -> [128,10]
                    outsb = obuf.tile([128, 8 * NCLS], F32, tag="outsb")
                    for h2 in range(2):
                        for g in range(4):
                            pt = tps.tile([128, NCLS], F32, tag="pt")
                            nc.tensor.transpose(
                                pt[:, 0:NCLS],
                                osb[32 * g:32 * g + NCLS,
                                    h2 * 128:h2 * 128 + 128],
                                ident[32 * g:32 * g + NCLS, 0:NCLS],
                                tile_position=(32 * g, 0),
                            )
                            nc.vector.tensor_copy(
                                outsb[:, (h2 * 4 + g) * NCLS:
                                      (h2 * 4 + g + 1) * NCLS],
                                pt[:, 0:NCLS])
                    dst = out_ap[t * 1024:(t + 1) * 1024, :].rearrange(
                        "(s2 g b) o -> b s2 g o", s2=2, g=4)
                    nc.scalar.dma_start(
                        out=dst,
                        in_=outsb[:, :].rearrange(
                            "p (s2 g o) -> p s2 g o", s2=2, g=4))

    nc.compile()
    return nc


def _host_constants(conv_w, W, b):
    msb = np.zeros((128, 3 * OUT), np.float32)
    for g in range(4):
        for dj in range(3):
            for i in range(OUT):
                for di in range(KW):
                    msb[32 * g + i + di, OUT * dj + i] = conv_w[di, dj]
    wsb = np.zeros((128, OUT * NCLS), np.float32)
    for blk in range(4):
        for i in range(OUT):
            for c in range(OUT):
                wsb[32 * blk + i, NCLS * c:NCLS * c + NCLS] = W[i * OUT + c, :]
    biasv = np.zeros((128, 1), np.float32)
    for g in range(4):
        biasv[32 * g:32 * g + NCLS, 0] = b
    ident = np.zeros((128, 32), np.float32)
    for p in range(128):
        ident[p, p % 32] = 1.0
    return msb, wsb, biasv, ident


def _run(x, conv_w, W, b, trace=False, mm_dtype=F32R):
    x = np.ascontiguousarray(np.asarray(x, dtype=np.float32))
    conv_w = np.asarray(conv_w, dtype=np.float32)
    W = np.asarray(W, dtype=np.float32)
    b = np.asarray(b, dtype=np.float32)
    assert x.shape == (B_TOTAL, NPIX), x.shape

    key = ("prog", str(mm_dtype))
    if key not in _CACHE:
        _CACHE[key] = _build_program(mm_dtype)
    nc = _CACHE[key]

    msb, wsb, biasv, ident = _host_constants(conv_w, W, b)
    in_maps = []
    for i in range(NCORES):
        in_maps.append({
            "x": x[i * B_CORE:(i + 1) * B_CORE],
            "msb": msb, "wsb": wsb, "biasv": biasv, "ident": ident,
        })
    res = run_bass_kernel_spmd(nc, in_maps, core_ids=list(range(NCORES)),
                               trace=trace)
    out = np.concatenate([res.results[i]["out"] for i in range(NCORES)], axis=0)
    return out, res


def kernel(x, conv_w, W, b):
    out, _ = _run(x, conv_w, W, b, trace=False)
    return out


# revision 11
# speedup vs baseline: 10.0448x; 10.0448x over previous
"""Trainium2 Bass kernel for DigitConvolutionalModel:
    out = relu(conv2d_3x3_valid(x.reshape(B,28,28))) .reshape(B,676) @ W + b

Strategy (pure data parallel over 8 cores, B=32768 -> 4096/core):

Per core, samples are processed in 8 "supergroups" (SG) of 512 = 4 groups
(g) of 128. The input DMA loads each group's 128 images in a
"row-on-partition" layout: SBUF X[32g + r, (b, c)] = x[b, 28r + c]
(28 image rows on partitions at 32-aligned bases, batch x column in the
free dim; 112-byte contiguous runs), then casts fp32 -> fp16 on ACT/DVE.

Conv (cross-correlation) becomes 3 PSUM-accumulated PE matmuls per
column-chunk: with the host-built Toeplitz matrices
M_dj[r, i] = conv_w[r-i, dj] (28x26 zero-padded to M=32),
    Y[i, (c, b)] = sum_dj  M_dj^T  @  X[:, (c+dj, b)]
which contracts the row dim r on partitions. Output lands already
"transposed" (output-row i on partitions, (c, b) in the free dim, c-major
so each c gives 128 contiguous batch columns). The small K=28/M=32
matmuls are packed into 32x32 PE tiles: group g reads row-group 32g and
writes col-group 32g (diagonal tiles; two row-groups feeding one
col-group crashes this toolchain's runtime, so the full 16-position
spread is not available). The 4 groups of one chunk fully cover the 128
PSUM partitions of one bank.

ReLU copies PSUM->SBUF (alternating DVE/ACT), giving H[32cg + i, ...]
with h transposed per sample (fp16). The FC layer contracts i per column:
    out^T[o, b] += W_c^T @ relu(Y)[:, (c, b)],   W_c[i, o] = W[26i+c, o]
as 26 PSUM-accumulated K=26/M=32(10 used)/N=256 matmuls per group. Bias
is added on DVE with a per-partition scalar, and small PE transposes flip
out^T[10,128] tiles into [128,10] for a contiguous store.

All matmul operands are fp16 (e5m10; values here are O(10) so well in
range); PSUM accumulation is fp32, so the only precision loss is the
~2^-11 input rounding.
"""

import sys
import numpy as np

for _p in ("/opt/trn_rl_repo", "/root/.axon_site/_ro/trn_rl_repo"):
    if _p not in sys.path:
        sys.path.insert(0, _p)

import concourse.bass as bass  # noqa: E402,F401
import concourse.tile as tile  # noqa: E402
from concourse import bacc, mybir  # noqa: E402
from concourse.bass_utils import run_bass_kernel_spmd  # noqa: E402

IMG = 28
KW = 3
OUT = 26  # IMG - KW + 1
NPIX = IMG * IMG          # 784
NOUTPIX = OUT * OUT       # 676
NCLS = 10
NCORES = 8
B_TOTAL = 32768
B_CORE = B_TOTAL // NCORES   # 4096
SG = 512                     # samples per supergroup (4 groups of 128)
N_SG = B_CORE // SG          # 8
NQ = 7                       # column chunks: 6x4 + 1x2 = 26 columns
HSTRIDE = OUT * 128          # 3328: per-supergroup h stride in h_pair
F32 = mybir.dt.float32
F32R = mybir.dt.float32r
F16 = mybir.dt.float16

_CACHE = {}


def _chunk_cols(q):
    """(first output column, n columns) of chunk q."""
    return 4 * q, (2 if q == NQ - 1 else 4)


def _build_program(mm_dtype=F16, n_sg=N_SG, rep=1):
    """Build + compile the per-core Bass program (identical on all cores)."""
    nc = bacc.Bacc("TRN2", target_bir_lowering=False, debug=False,
                   num_devices=NCORES)

    x_d = nc.dram_tensor("x", (B_CORE, NPIX), F32, kind="ExternalInput")
    msb_d = nc.dram_tensor("msb", (128, 3 * 32), mm_dtype, kind="ExternalInput")
    wsb_d = nc.dram_tensor("wsb", (128, OUT * 32), mm_dtype,
                           kind="ExternalInput")
    bias_d = nc.dram_tensor("biasv", (128, 1), F32, kind="ExternalInput")
    id_d = nc.dram_tensor("ident", (128, 32), F32, kind="ExternalInput")
    out_d = nc.dram_tensor("out", (B_CORE, NCLS), F32, kind="ExternalOutput")

    x_ap = x_d.ap()
    out_ap = out_d.ap()

    with tile.TileContext(nc) as tc:
        with (
            tc.tile_pool(name="consts", bufs=1) as consts,
            tc.tile_pool(name="xin", bufs=2) as xin,
            tc.tile_pool(name="hbuf", bufs=2) as hbuf,
            tc.tile_pool(name="obuf", bufs=2) as obuf,
            tc.tile_pool(name="convps", bufs=4, space="PSUM") as convps,
            tc.tile_pool(name="fcps", bufs=2, space="PSUM") as fcps,
            tc.tile_pool(name="tps", bufs=2, space="PSUM") as tps,
        ):
            msb = consts.tile([128, 3 * 32], mm_dtype)
            wsb = consts.tile([128, OUT * 32], mm_dtype)
            biasv = consts.tile([128, 1], F32)
            ident = consts.tile([128, 32], F32)
            nc.sync.dma_start(out=msb[:, :], in_=msb_d.ap())
            nc.sync.dma_start(out=wsb[:, :], in_=wsb_d.ap())
            nc.sync.dma_start(out=biasv[:, :], in_=bias_d.ap())
            nc.sync.dma_start(out=ident[:, :], in_=id_d.ap())

            h_pair = None
            for s in [s_ for _ in range(rep) for s_ in range(n_sg)]:
                # ---- load supergroup s: 4 groups of 128 samples ----
                xraw = xin.tile([128, 128 * IMG], F32, tag="xraw")
                if s < 2:
                    nc.vector.memset(xraw[:, :], 0.0)
                for g in range(4):
                    row0 = s * SG + g * 128
                    src = x_ap[row0:row0 + 128, :].rearrange(
                        "b (r c) -> r b c", r=IMG)
                    dst = xraw[32 * g:32 * g + IMG, :].rearrange(
                        "p (b c) -> p b c", c=IMG)
                    nc.sync.dma_start(out=dst, in_=src)
                # cast fp32 -> fp16 for the PE, split across ACT and DVE
                xt = xin.tile([128, 128 * IMG], mm_dtype, tag="xt")
                halfw = 128 * IMG // 2
                nc.scalar.activation(xt[:, 0:halfw], xraw[:, 0:halfw],
                                     mybir.ActivationFunctionType.Copy)
                nc.vector.tensor_copy(xt[:, halfw:], xraw[:, halfw:])

                if s % 2 == 0:
                    h_pair = hbuf.tile([128, 2 * HSTRIDE], mm_dtype, tag="h")
                    # free layout: s_half*3328 + c*128 + b   (3328 = 26*128)
                half = s % 2

                # ---- conv: per column-chunk q, 4 groups x 3 taps ----
                xv = [
                    xt[32 * g:32 * g + IMG, :].rearrange(
                        "p (b c) -> p c b", c=IMG)
                    for g in range(4)
                ]
                for q in range(NQ):
                    c0, ncol = _chunk_cols(q)
                    pq = convps.tile([128, 512], F32, tag="pq")
                    for g in range(4):
                        cg = g
                        for dj in range(3):
                            nc.tensor.matmul(
                                pq[32 * cg:32 * cg + 32, 0:ncol * 128],
                                msb[32 * g:32 * g + IMG, 32 * dj:32 * dj + 32],
                                xv[g][:, c0 + dj:c0 + dj + ncol, :],
                                start=(dj == 0), stop=(dj == 2),
                                tile_position=(32 * g, 32 * cg),
                            )
                    # ---- relu PSUM -> SBUF (h transposed, c-major) ----
                    hslice = h_pair[:, half * HSTRIDE + c0 * 128:
                                    half * HSTRIDE + (c0 + ncol) * 128]
                    if (s * NQ + q) % 2 == 0:
                        nc.vector.tensor_scalar_max(
                            hslice, pq[:, 0:ncol * 128], 0.0)
                    else:
                        nc.scalar.activation(
                            hslice, pq[:, 0:ncol * 128],
                            mybir.ActivationFunctionType.Relu)

                # ---- FC + bias + transpose + store, per SG pair ----
                if s % 2 == 1:
                    t = s // 2
                    ot = fcps.tile([128, 256], F32, tag="ot")
                    hv = [
                        h_pair[32 * rg:32 * rg + OUT, :].rearrange(
                            "p (s2 c b) -> p c s2 b", s2=2, b=128)
                        for rg in range(4)
                    ]
                    for g in range(4):
                        for c in range(OUT):
                            rg = g
                            nc.tensor.matmul(
                                ot[32 * g:32 * g + 32, 0:256],
                                wsb[32 * rg:32 * rg + OUT, 32 * c:32 * c + 32],
                                hv[rg][:, c, :, :],
                                start=(c == 0), stop=(c == OUT - 1),
                                tile_position=(32 * rg, 32 * g),
                            )
                    # bias add (per-partition scalar) PSUM -> SBUF
                    osb = obuf.tile([128, 256], F32, tag="osb")
                    nc.vector.tensor_scalar_add(osb[:, :], ot[:, :],
                                                biasv[:, 0:1])
                    # transpose out^T [10,128] tiles -> [128,10]
                    outsb = obuf.tile([128, 8 * NCLS], F32, tag="outsb")
                    for h2 in range(2):
                        for g in range(4):
                            pt = tps.tile([128, NCLS], F32, tag="pt")
                            nc.tensor.transpose(
                                pt[:, 0:NCLS],
                                osb[32 * g:32 * g + NCLS,
                                    h2 * 128:h2 * 128 + 128],
                                ident[32 * g:32 * g + NCLS, 0:NCLS],
                                tile_position=(32 * g, 0),
                            )
                            nc.vector.tensor_copy(
                                outsb[:, (h2 * 4 + g) * NCLS:
                                      (h2 * 4 + g + 1) * NCLS],
                                pt[:, 0:NCLS])
                    dst = out_ap[t * 1024:(t + 1) * 1024, :].rearrange(
                        "(s2 g b) o -> b s2 g o", s2=2, g=4)
                    nc.scalar.dma_start(
                        out=dst,
                        in_=outsb[:, :].rearrange(
                            "p (s2 g o) -> p s2 g o", s2=2, g=4))

    nc.compile()
    return nc


def _round_fp32r(a):
    """Round fp32 array to fp32r (e8m11: low 12 mantissa bits zero), RNE."""
    u = np.ascontiguousarray(a, dtype=np.float32).view(np.uint32)
    lsb = (u >> 12) & 1
    r = (u + 0x7FF + lsb) & np.uint32(0xFFFFF000)
    return r.view(np.float32)


def _host_constants(conv_w, W, b):
    """msb: Toeplitz conv matrices; wsb: FC weights; both M-padded to 32."""
    msb = np.zeros((128, 3 * 32), np.float32)
    for g in range(4):
        for dj in range(3):
            for i in range(OUT):
                for di in range(KW):
                    msb[32 * g + i + di, 32 * dj + i] = conv_w[di, dj]
    wsb = np.zeros((128, OUT * 32), np.float32)
    for blk in range(4):
        for i in range(OUT):
            for c in range(OUT):
                wsb[32 * blk + i, 32 * c:32 * c + NCLS] = W[i * OUT + c, :]
    biasv = np.zeros((128, 1), np.float32)
    for g in range(4):
        biasv[32 * g:32 * g + NCLS, 0] = b
    ident = np.zeros((128, 32), np.float32)
    for p in range(128):
        ident[p, p % 32] = 1.0
    return msb, wsb, biasv, ident


def _run(x, conv_w, W, b, trace=False, mm_dtype=F16):
    x = np.ascontiguousarray(np.asarray(x, dtype=np.float32))
    conv_w = np.asarray(conv_w, dtype=np.float32)
    W = np.asarray(W, dtype=np.float32)
    b = np.asarray(b, dtype=np.float32)
    assert x.shape == (B_TOTAL, NPIX), x.shape

    key = ("prog", str(mm_dtype))
    if key not in _CACHE:
        _CACHE[key] = _build_program(mm_dtype)
    nc = _CACHE[key]

    msb, wsb, biasv, ident = _host_constants(conv_w, W, b)
    if mm_dtype == F16:
        msb_r, wsb_r = msb.astype(np.float16), wsb.astype(np.float16)
    else:
        msb_r, wsb_r = msb, wsb
    in_maps = []
    for i in range(NCORES):
        in_maps.append({
            "x": x[i * B_CORE:(i + 1) * B_CORE],
            "msb": msb_r, "wsb": wsb_r, "biasv": biasv, "ident": ident,
        })
    res = run_bass_kernel_spmd(nc, in_maps, core_ids=list(range(NCORES)),
                               trace=trace)
    out = np.concatenate([res.results[i]["out"] for i in range(NCORES)],
                         axis=0)
    return out, res


def kernel(x, conv_w, W, b):
    out, _ = _run(x, conv_w, W, b, trace=False)
    return out
